# revision 20
# baseline (speedup 1.0000x reference)
"""Trainium2 Bass kernel for the DigitalTwinModel (3-layer LSTM digital twin).

Strategy: hybrid MP-4 x DP-2 in bf16.
  - The 8 cores form two replica groups {0..3} and {4..7}; each group owns a
    batch half (128 rows).  Within a group the hidden dim is sharded 4-way:
    core (g, r) owns hidden features r*256:(r+1)*256 of every LSTM layer's
    h/c state plus the matching 4x256 gate rows of W_ih/W_hh.
  - Everything is bf16 on the wire and in the PE (fp32 PSUM accumulate,
    fp32 elementwise/cell state): a 1/4 weight shard fits SBUF-resident,
    AllGather payloads halve ([1024,128] bf16 out = 256KB -> ~21.5us), and
    the PE runs at full rate at batch=128 free size.
  - 3 AllGathers per timestep (one per layer, 4-rank groups).  The decoder
    (Wd1+relu, then M = We@Wd2 which fuses the output projection with the
    re-encode) is replicated per core; out[:,t,:] is produced batch-major
    via matmul(lhsT=rT, rhs=Wd2^T) one step deferred inside the next AG0
    window, together with the W_hh prefetches into the gate PSUMs.
  - Gate PSUM is split per 2KB bank (pgXa = g,g,i,i / pgXb = f,f,o,o) so
    the elementwise chain can start as soon as the first bank's
    accumulation group closes instead of waiting for all 64 matmuls.
    Decoder PSUMs reuse the gate banks in dead lifetime windows.
  - Gather buffers use a [P, rank*(2B)] layout so DMA runs are 512B and
    dodge the sub-512B descriptor penalty.
  - Tunable filler matmuls on resident weights keep the PE p-state ramped
    through the collective windows so every critical-path matmul burst
    runs at full clock.
"""

import numpy as np
import ml_dtypes

import concourse.bass as bass
import concourse.mybir as mybir
from concourse import bacc
import concourse.tile as tile
from concourse.bass_utils import run_bass_kernel_spmd

F32 = mybir.dt.float32
BF16 = mybir.dt.bfloat16
AF = mybir.ActivationFunctionType

B, D_IN, H, L, T = 256, 512, 1024, 3, 32
NCORES = 8
GP = 4                     # ranks per replica group
NG = NCORES // GP          # replica groups (data-parallel)
BH = B // NG               # batch rows per group
P = 128
SH = H // GP               # hidden features owned per core (256)
KT_H = H // P              # 8 k-tiles over the hidden dim
MT_G = 4 * SH // P         # 8 m-tiles of gates per core
HB = MT_G // 2             # 4 m-tiles per PSUM bank
GROUPS = [[0, 1, 2, 3], [4, 5, 6, 7]]
# gate m-tile order: (g,g,i,i | f,f,o,o) -> bank A holds g,i; bank B f,o.
GATE_ORDER = [2, 0, 1, 3]  # torch gate chunks: i=0, f=1, g=2, o=3

# PE p-state filler matmuls per collective window (each ~213ns at 2.4GHz)
FILL = (100, 114, 140)


def _touch(nc, ap2d):
    """Tiny ldweights that makes the PE observe a tile's producer semaphore
    (fused matmuls have a single sync-wait slot)."""
    nc.tensor.ldweights(weights=ap2d[0:1, 0:2].bitcast(BF16))


def build_program(timesteps=T):
    nc = bacc.Bacc(None, num_devices=NCORES, dynamic_dma_scratch_size=2048)

    # ---- kernel I/O (per-core payloads supplied from the host) ----
    wih = [nc.dram_tensor(f"wih{l}", [H, 4 * SH], BF16, kind="ExternalInput") for l in range(L)]
    whh = [nc.dram_tensor(f"whh{l}", [H, 4 * SH], BF16, kind="ExternalInput") for l in range(L)]
    bg = [nc.dram_tensor(f"bg{l}", [1, 4 * SH], BF16, kind="ExternalInput") for l in range(L)]
    wd1 = nc.dram_tensor("wd1", [H, H], BF16, kind="ExternalInput")
    wm = nc.dram_tensor("wm", [H, H], BF16, kind="ExternalInput")
    wd2 = nc.dram_tensor("wd2", [H, D_IN], BF16, kind="ExternalInput")
    bd1 = nc.dram_tensor("bd1", [1, H], BF16, kind="ExternalInput")
    bm = nc.dram_tensor("bm", [1, H], BF16, kind="ExternalInput")
    enc0 = nc.dram_tensor("enc0", [H, BH], BF16, kind="ExternalInput")
    out = nc.dram_tensor("out", [BH, timesteps, D_IN], F32, kind="ExternalOutput")

    with tile.TileContext(nc) as tc:
        with (
            tc.tile_pool(name="singles", bufs=1) as singles,
            tc.tile_pool(name="acts", bufs=1) as acts,
            tc.tile_pool(name="gtmp", bufs=1) as gtmp,
            tc.tile_pool(name="hloc", bufs=2) as hlocp,
            tc.tile_pool(name="obuf", bufs=1) as obuf,
            tc.tile_pool(name="pg", bufs=1, space="PSUM") as pgp,
            tc.tile_pool(name="dram", bufs=2, space="DRAM") as dram,
        ):
            # ---- load resident weights/biases into SBUF ----
            s_wih, s_whh, s_bg = [], [], []
            for l in range(L):
                w = singles.tile([P, KT_H, 4 * SH], BF16, tag=f"swih{l}", name=f"swih{l}")
                nc.sync.dma_start(out=w, in_=wih[l][:].rearrange("(kk p) m -> p kk m", p=P))
                _touch(nc, w[:, 0, :])
                s_wih.append(w)
            for l in range(L):
                w = singles.tile([P, KT_H, 4 * SH], BF16, tag=f"swhh{l}", name=f"swhh{l}")
                nc.sync.dma_start(out=w, in_=whh[l][:].rearrange("(kk p) m -> p kk m", p=P))
                _touch(nc, w[:, 0, :])
                s_whh.append(w)
            for l in range(L):
                t_ = singles.tile([1, 4 * SH], BF16, tag=f"sbg{l}", name=f"sbg{l}")
                nc.sync.dma_start(out=t_, in_=bg[l][:])
                s_bg.append(t_)
            s_wd1 = singles.tile([P, KT_H, H], BF16, tag="swd1", name="swd1")
            nc.sync.dma_start(out=s_wd1, in_=wd1[:].rearrange("(kk p) m -> p kk m", p=P))
            _touch(nc, s_wd1[:, 0, :])
            s_wm = singles.tile([P, KT_H, H], BF16, tag="swm", name="swm")
            nc.sync.dma_start(out=s_wm, in_=wm[:].rearrange("(kk p) m -> p kk m", p=P))
            _touch(nc, s_wm[:, 0, :])
            s_wd2 = singles.tile([P, KT_H, D_IN], BF16, tag="swd2", name="swd2")
            nc.sync.dma_start(out=s_wd2, in_=wd2[:].rearrange("(kk p) m -> p kk m", p=P))
            _touch(nc, s_wd2[:, 0, :])
            s_bd1 = singles.tile([1, H], BF16, tag="sbd1", name="sbd1")
            nc.sync.dma_start(out=s_bd1, in_=bd1[:])
            s_bm = singles.tile([1, H], BF16, tag="sbm", name="sbm")
            nc.sync.dma_start(out=s_bm, in_=bm[:])
            ones = singles.tile([1, BH], BF16, tag="ones", name="ones")
            nc.vector.memset(ones, 1.0)

            s_c = []
            for l in range(L):
                c = singles.tile([P, 2, BH], F32, tag=f"c{l}", name=f"c{l}")
                nc.vector.memset(c, 0.0)
                s_c.append(c)

            def pghalf(tag, name):
                return pgp.tile([P, HB, BH], F32, tag=tag, name=name)

            def bias_fold(pa, pb, btile):
                """Open both banks' accumulation groups; add per-(partition,
                m-tile) biases via rank-1 ones matmuls. start=True on the
                bank-first m-tile clears the whole 2KiB bank."""
                for half, pt in ((0, pa), (1, pb)):
                    for m in range(HB):
                        nc.tensor.matmul(
                            pt[:, m, :],
                            lhsT=btile[0:1, (half * HB + m) * P:(half * HB + m + 1) * P],
                            rhs=ones[0:1, :],
                            start=(m == 0),
                            stop=False,
                        )

            def rhs_kt(hT, kk):
                """k-tile kk of a gathered tensor in [P, GP, 2*BH]... note:
                hT here is [P, GP, 2, BH]; kk maps to (rank, half)."""
                return hT[:, kk // 2, kk % 2, :]

            def mm_gates(pa, pb, w, hT, close):
                """Accumulate w^T @ hT into the two bank tiles, bank-A m-tiles
                first so elementwise can start while bank B accumulates."""
                for half, pt in ((0, pa), (1, pb)):
                    for m in range(HB):
                        for kk in range(KT_H):
                            nc.tensor.matmul(
                                pt[:, m, :],
                                lhsT=w[:, kk, (half * HB + m) * P:(half * HB + m + 1) * P],
                                rhs=rhs_kt(hT, kk),
                                start=False,
                                stop=(close and kk == KT_H - 1 and m == HB - 1),
                            )

            def mm_dense(pa, pb, w, xa, xb, close, kk_outer=False):
                """Like mm_gates but rhs is a local tensor split into two
                [P, HB, BH] half tiles. kk_outer=True consumes the halves
                incrementally (for chains where they become ready in order)."""
                def xkt(kk):
                    return (xa if kk < HB else xb)[:, kk % HB, :]
                if kk_outer:
                    for phase in range(2):
                        for kk in range(phase * HB, (phase + 1) * HB):
                            for half, pt in ((0, pa), (1, pb)):
                                for m in range(HB):
                                    nc.tensor.matmul(
                                        pt[:, m, :],
                                        lhsT=w[:, kk, (half * HB + m) * P:(half * HB + m + 1) * P],
                                        rhs=xkt(kk),
                                        start=False,
                                        stop=(close and kk == KT_H - 1 and m == HB - 1),
                                    )
                else:
                    for half, pt in ((0, pa), (1, pb)):
                        for m in range(HB):
                            for kk in range(KT_H):
                                nc.tensor.matmul(
                                    pt[:, m, :],
                                    lhsT=w[:, kk, (half * HB + m) * P:(half * HB + m + 1) * P],
                                    rhs=xkt(kk),
                                    start=False,
                                    stop=(close and kk == KT_H - 1 and m == HB - 1),
                                )

            def elementwise(l, pa, pb):
                """bank A = (g,g,i,i), bank B = (f,f,o,o) -> h'_l (bf16)."""
                tg = gtmp.tile([P, 2, BH], F32, tag="tg", name="tg")
                si = gtmp.tile([P, 2, BH], F32, tag="si", name="si")
                sf = gtmp.tile([P, 2, BH], F32, tag="sf", name="sf")
                so = gtmp.tile([P, 2, BH], F32, tag="so", name="so")
                t1 = gtmp.tile([P, 2, BH], F32, tag="t1", name="t1")
                t2 = gtmp.tile([P, 2, BH], F32, tag="t2", name="t2")
                tc_ = gtmp.tile([P, 2, BH], F32, tag="tc", name="tc")
                nc.scalar.activation(tg, pa[:, 0:2, :], AF.Tanh)
                nc.scalar.activation(si, pa[:, 2:4, :], AF.Sigmoid)
                nc.vector.tensor_mul(t1, si, tg)                   # i * g
                nc.scalar.activation(sf, pb[:, 0:2, :], AF.Sigmoid)
                nc.vector.tensor_mul(t2, sf, s_c[l])               # f * c
                nc.scalar.activation(so, pb[:, 2:4, :], AF.Sigmoid)
                nc.vector.tensor_add(s_c[l], t1, t2)
                nc.scalar.activation(tc_, s_c[l], AF.Tanh)
                hl = hlocp.tile([P, 2, BH], BF16, tag=f"hl{l}", name=f"hl{l}")
                nc.vector.tensor_mul(hl, so, tc_)
                return hl

            def allgather(hl, l):
                # agin row p = [c0 batch..., c1 batch...]: 512B contiguous
                agin = dram.tile([P, SH // P * BH], BF16, tag=f"agin{l}", name=f"agin{l}")
                agout = dram.tile([GP * P, SH // P * BH], BF16, tag=f"agout{l}", name=f"agout{l}")
                nc.sync.dma_start(out=agin, in_=hl[:].rearrange("p c b -> p (c b)"))
                nc.gpsimd.collective_compute(
                    "AllGather",
                    mybir.AluOpType.bypass,
                    replica_groups=GROUPS,
                    ins=[agin.opt()],
                    outs=[agout.opt()],
                )
                return agout

            def fetch_hT(agout, l):
                # [P, rank, half, BH]; in-side rows are 512B contiguous
                hT = acts.tile([P, GP, 2, BH], BF16, tag=f"hT{l}", name=f"hT{l}")
                nc.sync.dma_start(
                    out=hT[:].rearrange("p r c b -> p r (c b)"),
                    in_=agout[:].rearrange("(r p) x -> p r x", p=P))
                return hT

            def emit_outbt(rTa, rTb, tstep):
                """out[:, tstep, :] = (rT^T @ Wd2^T); bd2 added on host."""
                _touch(nc, rTa[:, 0, :])
                _touch(nc, rTb[:, 0, :])
                po = pghalf("pg0a", "po")
                pov = po[:].rearrange("p a b -> p (a b)")
                for kk in range(KT_H):
                    nc.tensor.matmul(
                        pov,
                        lhsT=(rTa if kk < HB else rTb)[:, kk % HB, :],
                        rhs=s_wd2[:, kk, :],
                        start=kk == 0,
                        stop=kk == KT_H - 1,
                    )
                ob = obuf.tile([P, D_IN], F32, tag="ob", name="ob")
                nc.vector.tensor_copy(out=ob, in_=pov)
                nc.sync.dma_start(out=out[:, tstep, :], in_=ob)

            # ---- PE p-state filler ----
            pfill = pgp.tile([P, D_IN], F32, tag="fill", name="pfill")

            def fill(n):
                for _ in range(n):
                    nc.tensor.matmul(
                        pfill,
                        lhsT=s_wd1[:, 0, 0:P],
                        rhs=s_wd1[:, 1, 0:D_IN],
                        start=True,
                        stop=True,
                        skip_group_check=True,
                    )

            # ---- prologue: enc(0); open gate groups (h(-1) = 0) ----
            enca = acts.tile([P, HB, BH], BF16, tag="enca", name="enca")
            encb = acts.tile([P, HB, BH], BF16, tag="encb", name="encb")
            nc.sync.dma_start(
                out=enca, in_=enc0[0:HB * P, :].rearrange("(kk p) b -> p kk b", p=P))
            nc.sync.dma_start(
                out=encb, in_=enc0[HB * P:H, :].rearrange("(kk p) b -> p kk b", p=P))
            pga = [None] * L
            pgb = [None] * L
            for l in range(L):
                pga[l] = pghalf(f"pg{l}a", f"pg{l}a")
                pgb[l] = pghalf(f"pg{l}b", f"pg{l}b")
                bias_fold(pga[l], pgb[l], s_bg[l])

            hT = [None] * L
            rT_prev, t_prev = None, None

            for t in range(timesteps):
                # ---- layer 0 gates (enc from prologue / dec chain) ----
                mm_dense(pga[0], pgb[0], s_wih[0], enca, encb, close=True,
                         kk_outer=(t > 0))
                h0l = elementwise(0, pga[0], pgb[0])
                ag0 = allgather(h0l, 0)
                # AG0 window: deferred out-write, W_hh1/W_hh2 prefetch
                if t > 0:
                    emit_outbt(rT_prev[0], rT_prev[1], t_prev)
                    _touch(nc, hT[1][:, 0, 0, :])
                    pga[1], pgb[1] = pghalf("pg1a", "pg1a"), pghalf("pg1b", "pg1b")
                    bias_fold(pga[1], pgb[1], s_bg[1])
                    mm_gates(pga[1], pgb[1], s_whh[1], hT[1], close=False)
                    _touch(nc, hT[2][:, 0, 0, :])
                    pga[2], pgb[2] = pghalf("pg2a", "pg2a"), pghalf("pg2b", "pg2b")
                    bias_fold(pga[2], pgb[2], s_bg[2])
                    mm_gates(pga[2], pgb[2], s_whh[2], hT[2], close=False)
                fill(FILL[0])
                hT[0] = fetch_hT(ag0, 0)

                # ---- layer 1 ----
                _touch(nc, hT[0][:, 0, 0, :])
                mm_gates(pga[1], pgb[1], s_wih[1], hT[0], close=True)
                h1l = elementwise(1, pga[1], pgb[1])
                ag1 = allgather(h1l, 1)
                # AG1 window: W_hh0 prefetch for t+1
                if t + 1 < timesteps:
                    pga[0], pgb[0] = pghalf("pg0a", "pg0a"), pghalf("pg0b", "pg0b")
                    bias_fold(pga[0], pgb[0], s_bg[0])
                    mm_gates(pga[0], pgb[0], s_whh[0], hT[0], close=False)
                fill(FILL[1])
                hT[1] = fetch_hT(ag1, 1)

                # ---- layer 2 ----
                _touch(nc, hT[1][:, 0, 0, :])
                mm_gates(pga[2], pgb[2], s_wih[2], hT[1], close=True)
                h2l = elementwise(2, pga[2], pgb[2])
                ag2 = allgather(h2l, 2)
                fill(FILL[2])
                hT[2] = fetch_hT(ag2, 2)

                # ---- decoder chain: d1 -> rT -> (M -> enc -> G0) ----
                _touch(nc, hT[2][:, 0, 0, :])
                pd1a, pd1b = pghalf("pg1a", "pd1a"), pghalf("pg1b", "pd1b")
                bias_fold(pd1a, pd1b, s_bd1)
                mm_gates(pd1a, pd1b, s_wd1, hT[2], close=True)
                rTa = acts.tile([P, HB, BH], BF16, tag="rTa", name="rTa")
                rTb = acts.tile([P, HB, BH], BF16, tag="rTb", name="rTb")
                nc.scalar.activation(rTa, pd1a, AF.Relu)
                nc.scalar.activation(rTb, pd1b, AF.Relu)

                if t + 1 < timesteps:
                    _touch(nc, rTa[:, 0, :])
                    pea, peb = pghalf("pg2a", "pea"), pghalf("pg2b", "peb")
                    bias_fold(pea, peb, s_bm)
                    _touch(nc, rTb[:, 0, :])
                    mm_dense(pea, peb, s_wm, rTa, rTb, close=True, kk_outer=True)
                    enca = acts.tile([P, HB, BH], BF16, tag="enca", name="enca")
                    encb = acts.tile([P, HB, BH], BF16, tag="encb", name="encb")
                    nc.scalar.activation(enca, pea, AF.Relu)
                    nc.scalar.activation(encb, peb, AF.Relu)
                    rT_prev, t_prev = (rTa, rTb), t
                else:
                    emit_outbt(rTa, rTb, t)

    nc.compile()
    return nc


_CACHE = {}


def _get_program(timesteps):
    if timesteps not in _CACHE:
        _CACHE[timesteps] = build_program(timesteps)
    return _CACHE[timesteps]


def _prep_inputs(x, We, be, W_ih, W_hh, b_ih, b_hh, Wd1, bd1, Wd2, bd2):
    """Host-side layout: bf16 weights, per-core gate-row shards, folded
    decoder matrix M = We@Wd2, batch halves per replica group."""
    f = np.float32
    bf = ml_dtypes.bfloat16
    x, We, be = np.asarray(x, f), np.asarray(We, f), np.asarray(be, f)
    W_ih, W_hh = np.asarray(W_ih, f), np.asarray(W_hh, f)
    b_ih, b_hh = np.asarray(b_ih, f), np.asarray(b_hh, f)
    Wd1, bd1 = np.asarray(Wd1, f), np.asarray(bd1, f)
    Wd2, bd2 = np.asarray(Wd2, f), np.asarray(bd2, f)

    enc0T = np.maximum(x @ We.T + be, 0.0).T          # [H, B]
    M = We @ Wd2                                       # [H, H]
    bM = We @ bd2 + be                                 # [H]

    wd1T = np.ascontiguousarray(Wd1.T).astype(bf)
    wmT = np.ascontiguousarray(M.T).astype(bf)
    wd2T = np.ascontiguousarray(Wd2.T).astype(bf)
    bd1c = np.ascontiguousarray(bd1[None, :]).astype(bf)
    bmc = np.ascontiguousarray(bM[None, :]).astype(bf)

    in_maps = []
    for k in range(NCORES):
        g, r = k // GP, k % GP
        rows = np.concatenate(
            [np.arange(q * H + r * SH, q * H + (r + 1) * SH) for q in GATE_ORDER]
        )
        m = {
            "wd1": wd1T, "wm": wmT, "wd2": wd2T, "bd1": bd1c, "bm": bmc,
            "enc0": np.ascontiguousarray(enc0T[:, g * BH:(g + 1) * BH]).astype(bf),
        }
        for l in range(L):
            m[f"wih{l}"] = np.ascontiguousarray(W_ih[l][rows, :].T).astype(bf)
            m[f"whh{l}"] = np.ascontiguousarray(W_hh[l][rows, :].T).astype(bf)
            bsum = (b_ih[l] + b_hh[l])[rows]
            m[f"bg{l}"] = np.ascontiguousarray(bsum[None, :]).astype(bf)
        in_maps.append(m)
    return in_maps, bd2


def kernel(x, We, be, W_ih, W_hh, b_ih, b_hh, Wd1, bd1, Wd2, bd2, timesteps, **run_kw):
    tsteps = int(timesteps)
    nc = _get_program(tsteps)
    in_maps, bd2_np = _prep_inputs(x, We, be, W_ih, W_hh, b_ih, b_hh, Wd1, bd1, Wd2, bd2)
    res = run_bass_kernel_spmd(nc, in_maps, core_ids=list(range(NCORES)), **run_kw)
    kernel.last_results = res
    halves = [np.asarray(res.results[g * GP]["out"], np.float32) for g in range(NG)]
    out = np.concatenate(halves, axis=0) + bd2_np[None, None, :]
    return out
